# revision 19
# baseline (speedup 1.0000x reference)
"""Trainium2 Bass kernel for CARE position encoding (rotor sandwich).

out = R x R~ factorizes into 4 sequential Givens stages (blades 6,9,5,3).
Implementation highlights:
  - all cos/sin tables computed on the HOST and shipped fp16; the device
    does no transcendental work and never sees `pos`;
  - x stored per-core position-innermost: X[partition, slot*J + j]
    (J=256, 14 slots; multivector comps 0/15 are invariant -> host copy);
  - every stage = 4 fp16 DVE tensor_tensor ops (merged T multiply, two
    half U multiplies, merged add) whose innermost dims are 256-long
    unit-stride runs -> DVE 2x_1P packed mode;
  - slot permutation chosen so each plane's 8 rotated cells form a 2-dim
    slot lattice {c0+a*i+d*k} (one full-width T/A op) and stage 6's cells
    are exactly slots 0..7, so the x DMA splits into an early gating
    chunk and the rest overlaps stage-6 compute;
  - stage-3 (last) add is split by lattice k-halves so output DMAs start
    while the second half computes.
"""
import numpy as np

import concourse.bass as bass
import concourse.tile as tile
from concourse import bacc, mybir
from concourse.bass_utils import run_bass_kernel_spmd

F16 = mybir.dt.float16
F32 = mybir.dt.float32

P = 128
NCORES = 8
B, L, MV = 16, 16384, 16
MAX_LEN = 16384
ROWS_PER_CORE = B // NCORES          # 2
N = ROWS_PER_CORE * L                # 32768 positions per core
J = N // P                           # 256 positions per partition
NSLOT = 14

PLANE_BLADES = (3, 5, 9, 6)          # reference arg order
STAGE_ORDER = (6, 9, 5, 3)           # innermost rotor applied first

# slot[comp]; comps 0 and 15 bypass the device (host passthrough)
SLOT = {1: 12, 2: 2, 3: 4, 4: 6, 5: 0, 6: 10, 7: 8, 8: 9,
        9: 11, 10: 1, 11: 7, 12: 5, 13: 3, 14: 13}
COMPS = [c for c in range(MV) if c not in (0, 15)]
SLOT_TO_COMP = {s: c for c, s in SLOT.items()}

# Per-plane merged spec: lat = (c0, a, d): cells {c0+a*i+d*k, i<4, k<2},
# T-tile col = 2i+k.  usubs: U-op (q,e) iteration: out col, partner-read
# slot, tau(e=0) per q.  All offsets/steps in SLOT units (scaled by J).
PLANE_SPECS = {
    6: dict(lat=(0, 1, 4), usubs=[
        dict(u_off=4, u_dims=[[-4, 2], [1, 2]],
             xp_off=6, xp_dims=[[-2, 2], [-4, 2]], tau=(1, -1)),
        dict(u_off=2, u_dims=[[4, 2], [1, 2]],
             xp_off=5, xp_dims=[[2, 2], [-4, 2]], tau=(1, -1))]),
    9: dict(lat=(0, 4, 1), usubs=[
        dict(u_off=6, u_dims=[[-4, 2], [-1, 2]],
             xp_off=9, xp_dims=[[-8, 2], [3, 2]], tau=(1, -1)),
        dict(u_off=3, u_dims=[[4, 2], [-3, 2]],
             xp_off=0, xp_dims=[[8, 2], [5, 2]], tau=(1, -1))]),
    5: dict(lat=(4, 1, 6), usingle=dict(
        u_off=1, u_dims=[[2, 4], [-1, 2]],
        xp_off=4, xp_dims=[[1, 4], [6, 2]],
        ss_off=0, ss_dims=[[0, 4], [1, 2]]), usubs=[]),
    3: dict(lat=(0, 1, 10), usingle=dict(
        u_off=0, u_dims=[[2, 4], [1, 2]],
        xp_off=10, xp_dims=[[1, 4], [-10, 2]],
        ss_off=0, ss_dims=[[1, 4], [4, 2]]), usubs=[]),
}

# tables: per plane CC [J] + SS 4 rows [s,-s,-s,s]; stage order
_TBL_CC = {6: 0, 9: 5, 5: 10, 3: 15}
TBL_J = 24
# m3 SS has 8 rows: [t_q*s]_q + [-t_q*s]_q with t=(+,-,-,+)
_M3_TAU = (1.0, -1.0, -1.0, 1.0)

EARLY_OUT = (4, 10)                  # slots 4..9 final after stage 5


def _build_cayley(k=4):
    n = 1 << k
    C = np.zeros((n, n, n), dtype=np.float32)
    for a in range(n):
        for b in range(n):
            s, t = 0, a >> 1
            while t:
                s += bin(t & b).count("1")
                t >>= 1
            C[a, b, a ^ b] = -1.0 if (s & 1) else 1.0
    return C


def _verify_layout(cayley):
    """Re-derive every stage from SLOT/PLANE_SPECS and check against the
    runtime Cayley tensor via a tiny numeric simulation."""
    rng = np.random.default_rng(3)
    Jt = 8
    x = rng.standard_normal((MV, Jt))
    ang = rng.standard_normal((4, Jt))
    ref = x.copy()
    for si, m in enumerate(STAGE_ORDER):
        c2, s2 = np.cos(ang[si]), np.sin(ang[si])
        new = ref.copy()
        for a in range(MV):
            if bin(a & m).count("1") % 2 == 1:
                b = a ^ m
                new[a] = c2 * ref[a] + cayley[a, m, b] * s2 * ref[b]
        ref = new
    X = np.zeros((NSLOT, Jt))
    for c in COMPS:
        X[SLOT[c]] = x[c]
    for si, m in enumerate(STAGE_ORDER):
        c2, s2 = np.cos(ang[si]), np.sin(ang[si])
        sp = PLANE_SPECS[m]
        c0, a, d = sp["lat"]
        T = np.zeros((8, Jt))
        U = np.zeros((8, Jt))
        for i in range(4):
            for k in range(2):
                T[2 * i + k] = X[c0 + a * i + d * k] * c2
        if "usingle" in sp:
            us = sp["usingle"]
            taus = _M3_TAU if m == 3 else (1.0,) * 4
            for q in range(4):
                for e in range(2):
                    ucol = us["u_off"] + us["u_dims"][0][0] * q + \
                        us["u_dims"][1][0] * e
                    xs = us["xp_off"] + us["xp_dims"][0][0] * q + \
                        us["xp_dims"][1][0] * e
                    sgn = taus[q] * (1.0 if e == 0 else -1.0)
                    U[ucol] = X[xs] * sgn * s2
        for us in sp["usubs"]:
            for q in range(2):
                for e in range(2):
                    ucol = us["u_off"] + us["u_dims"][0][0] * q + \
                        us["u_dims"][1][0] * e
                    xs = us["xp_off"] + us["xp_dims"][0][0] * q + \
                        us["xp_dims"][1][0] * e
                    sgn = us["tau"][q] * (1.0 if e == 0 else -1.0)
                    U[ucol] = X[xs] * sgn * s2
        for i in range(4):
            for k in range(2):
                X[c0 + a * i + d * k] = T[2 * i + k] + U[2 * i + k]
    got = np.zeros((MV, Jt))
    got[0], got[15] = x[0], x[15]
    for c in COMPS:
        got[c] = X[SLOT[c]]
    assert np.abs(got - ref).max() < 1e-9, "layout/spec validation failed"


def _ap(base_ap, extra_off, dims):
    ap = [list(base_ap.ap[0])] + [list(d) for d in dims]
    return bass.AP(base_ap.tensor, base_ap.offset + extra_off, ap)


def _ss_dims(tau):
    """AP (offset_J, dims) into 4-row table [s,-s,-s,s] giving
    tau[q]*(-1)^e across (q,e)."""
    if tau == (1, 1):
        return 0, [[0, 2], [1, 2]]
    if tau == (-1, -1):
        return 1, [[0, 2], [-1, 2]]
    if tau == (1, -1):
        return 0, [[2, 2], [1, 2]]
    # (-1, 1): r = 1 - q + 2e
    return 1, [[-1, 2], [2, 2]]


def _build_program():
    nc = bacc.Bacc("TRN2", target_bir_lowering=False, debug=False,
                   enable_asserts=False, num_devices=NCORES)
    x_d = nc.dram_tensor("x", [P, NSLOT * J], F16, kind="ExternalInput")
    t_d = nc.dram_tensor("tbl", [P, TBL_J * J], F16, kind="ExternalInput")
    out_d = nc.dram_tensor("out", [P, NSLOT * J], F16, kind="ExternalOutput")

    with tile.TileContext(nc) as tc:
        with tc.tile_pool(name="data", bufs=1) as dpool, \
             tc.tile_pool(name="tu", bufs=1) as tupool:
            TBL = dpool.tile([P, TBL_J * J], F16)
            X = dpool.tile([P, NSLOT * J], F16)

            # stage-6 gate = its tables + x slots 0..7 only; the rest of x
            # and the later tables stream in under stage-6 compute
            nc.sync.dma_start(TBL[:, :5 * J], t_d[:, :5 * J])
            nc.sync.dma_start(X[:, :8 * J], x_d[:, :8 * J])
            nc.sync.dma_start(X[:, 8 * J:], x_d[:, 8 * J:])
            nc.sync.dma_start(TBL[:, 5 * J:], t_d[:, 5 * J:])

            tu_dims = [[2 * J, 4], [J, 2], [1, J]]
            for m in STAGE_ORDER:
                sp = PLANE_SPECS[m]
                c0, a, d = sp["lat"]
                cc_j = _TBL_CC[m]
                grid = [[a * J, 4], [d * J, 2], [1, J]]
                T = tupool.tile([P, 8 * J], F16, tag="t")
                U = tupool.tile([P, 8 * J], F16, tag="u")
                nc.vector.tensor_mul(
                    _ap(T[:], 0, tu_dims),
                    _ap(X[:], c0 * J, grid),
                    _ap(TBL[:], cc_j * J, [[0, 4], [0, 2], [1, J]]))
                if "usingle" in sp:
                    us = sp["usingle"]
                    nc.vector.tensor_mul(
                        _ap(U[:], us["u_off"] * J,
                            [[us["u_dims"][0][0] * J, 4],
                             [us["u_dims"][1][0] * J, 2], [1, J]]),
                        _ap(X[:], us["xp_off"] * J,
                            [[us["xp_dims"][0][0] * J, 4],
                             [us["xp_dims"][1][0] * J, 2], [1, J]]),
                        _ap(TBL[:], (cc_j + 1 + us["ss_off"]) * J,
                            [[us["ss_dims"][0][0] * J, 4],
                             [us["ss_dims"][1][0] * J, 2], [1, J]]))
                for us in sp["usubs"]:
                    so, sd = _ss_dims(tuple(us["tau"]))
                    nc.vector.tensor_mul(
                        _ap(U[:], us["u_off"] * J,
                            [[us["u_dims"][0][0] * J, 2],
                             [us["u_dims"][1][0] * J, 2], [1, J]]),
                        _ap(X[:], us["xp_off"] * J,
                            [[us["xp_dims"][0][0] * J, 2],
                             [us["xp_dims"][1][0] * J, 2], [1, J]]),
                        _ap(TBL[:], (cc_j + 1 + so) * J,
                            [[sd[0][0] * J, 2], [sd[1][0] * J, 2], [1, J]]))
                if m == STAGE_ORDER[-1]:
                    # split add by k-halves; overlap output DMAs
                    for k in range(2):
                        nc.vector.tensor_add(
                            _ap(X[:], (c0 + d * k) * J,
                                [[a * J, 4], [1, J]]),
                            _ap(T[:], k * J, [[2 * J, 4], [1, J]]),
                            _ap(U[:], k * J, [[2 * J, 4], [1, J]]))
                        lo = c0 + d * k
                        nc.sync.dma_start(
                            out_d[:, lo * J:(lo + 4) * J],
                            X[:, lo * J:(lo + 4) * J])
                else:
                    nc.vector.tensor_add(
                        _ap(X[:], c0 * J, grid),
                        _ap(T[:], 0, tu_dims),
                        _ap(U[:], 0, tu_dims))
                if m == 5:
                    aa, bb = EARLY_OUT
                    nc.sync.dma_start(out_d[:, aa * J:bb * J],
                                      X[:, aa * J:bb * J])

    nc.compile()
    return nc


_PROGRAM_CACHE = {}


def _get_program():
    if "p" not in _PROGRAM_CACHE:
        _PROGRAM_CACHE["p"] = _build_program()
    return _PROGRAM_CACHE["p"]


def _build_in_maps(x, pos, coefs, theta0, cayley):
    """Host-side: slot-permuted fp16 x + per-core cos/sin tables."""
    _verify_layout(cayley)
    ang = theta0.astype(np.float64) * np.asarray(coefs, np.float64)[None, :]
    ctab = np.cos(ang).astype(np.float16)          # (MAX_LEN, 4)
    stab = np.sin(ang).astype(np.float16)
    plane_idx = {m: PLANE_BLADES.index(m) for m in STAGE_ORDER}

    pos_i = np.clip(pos, 0, MAX_LEN - 1).astype(np.int64)
    comp_order = [SLOT_TO_COMP[s] for s in range(NSLOT)]

    in_maps = []
    for g in range(NCORES):
        xr = np.ascontiguousarray(
            x[g * ROWS_PER_CORE:(g + 1) * ROWS_PER_CORE]
        ).reshape(P, J, MV)
        xs = xr.transpose(0, 2, 1)[:, comp_order, :]       # (P, 14, J)
        x16 = np.ascontiguousarray(xs).astype(np.float16).reshape(
            P, NSLOT * J)

        pg = pos_i[g * ROWS_PER_CORE:(g + 1) * ROWS_PER_CORE].reshape(P, J)
        tbl = np.empty((P, TBL_J, J), dtype=np.float16)
        for m in STAGE_ORDER:
            cc = _TBL_CC[m]
            i = plane_idx[m]
            c2 = ctab[pg, i]
            s2 = stab[pg, i]
            tbl[:, cc, :] = c2
            if m == 3:
                for q in range(4):
                    tbl[:, cc + 1 + q, :] = np.float16(_M3_TAU[q]) * s2
                    tbl[:, cc + 5 + q, :] = np.float16(-_M3_TAU[q]) * s2
            else:
                tbl[:, cc + 1, :] = s2
                tbl[:, cc + 2, :] = -s2
                tbl[:, cc + 3, :] = -s2
                tbl[:, cc + 4, :] = s2
        in_maps.append({"x": x16, "tbl": tbl.reshape(P, TBL_J * J)})
    return in_maps


def kernel(x, pos, bx, by, bz, bw, theta, cayley, biv_mask, scalar_mask):
    x = np.asarray(x, dtype=np.float32)
    pos = np.asarray(pos)
    theta = np.asarray(theta, dtype=np.float32)
    cayley = np.asarray(cayley, dtype=np.float32)
    assert x.shape == (B, L, MV) and pos.shape == (B, L)

    coefs = [float(np.asarray(c, dtype=np.float32).reshape(MV)[b])
             for c, b in zip((bx, by, bz, bw), PLANE_BLADES)]
    theta0 = theta.reshape(MAX_LEN, 4)

    nc = _get_program()
    in_maps = _build_in_maps(x, pos, coefs, theta0, cayley)
    res = run_bass_kernel_spmd(nc, in_maps, core_ids=list(range(NCORES)))

    out = np.empty((B, L, MV), dtype=np.float32)
    comp_order = [SLOT_TO_COMP[s] for s in range(NSLOT)]
    for g in range(NCORES):
        r = res.results[g]["out"].reshape(P, NSLOT, J).astype(np.float32)
        og = np.empty((P, MV, J), dtype=np.float32)
        og[:, comp_order, :] = r
        xr = np.ascontiguousarray(
            x[g * ROWS_PER_CORE:(g + 1) * ROWS_PER_CORE]).reshape(P, J, MV)
        og[:, 0, :] = xr[:, :, 0]
        og[:, 15, :] = xr[:, :, 15]
        out[g * ROWS_PER_CORE:(g + 1) * ROWS_PER_CORE] = \
            og.transpose(0, 2, 1).reshape(ROWS_PER_CORE, L, MV)
    return out


# revision 20
# speedup vs baseline: 1.0332x; 1.0332x over previous
"""Trainium2 Bass kernel for CARE position encoding (rotor sandwich).

out = R x R~ factorizes into 4 sequential Givens stages (blades 6,9,5,3).
Implementation highlights:
  - all cos/sin tables computed on the HOST and shipped fp16; the device
    does no transcendental work and never sees `pos`;
  - x stored per-core position-innermost: X[partition, slot*J + j]
    (J=256, 14 slots; multivector comps 0/15 are invariant -> host copy);
  - every stage = 4 fp16 DVE tensor_tensor ops (merged T multiply, two
    half U multiplies, merged add) whose innermost dims are 256-long
    unit-stride runs -> DVE 2x_1P packed mode;
  - slot permutation chosen so each plane's 8 rotated cells form a 2-dim
    slot lattice {c0+a*i+d*k} (one full-width T/A op) and stage 6's cells
    are exactly slots 0..7, so the x DMA splits into an early gating
    chunk and the rest overlaps stage-6 compute;
  - stage-3 (last) add is split by lattice k-halves so output DMAs start
    while the second half computes.
"""
import numpy as np

import concourse.bass as bass
import concourse.tile as tile
from concourse import bacc, mybir
from concourse.bass_utils import run_bass_kernel_spmd

F16 = mybir.dt.float16
F32 = mybir.dt.float32

P = 128
NCORES = 8
B, L, MV = 16, 16384, 16
MAX_LEN = 16384
ROWS_PER_CORE = B // NCORES          # 2
N = ROWS_PER_CORE * L                # 32768 positions per core
J = N // P                           # 256 positions per partition
NSLOT = 14

PLANE_BLADES = (3, 5, 9, 6)          # reference arg order
STAGE_ORDER = (6, 9, 5, 3)           # innermost rotor applied first

# slot[comp]; comps 0 and 15 bypass the device (host passthrough)
SLOT = {1: 12, 2: 2, 3: 4, 4: 6, 5: 0, 6: 10, 7: 8, 8: 9,
        9: 11, 10: 1, 11: 7, 12: 5, 13: 3, 14: 13}
COMPS = [c for c in range(MV) if c not in (0, 15)]
SLOT_TO_COMP = {s: c for c, s in SLOT.items()}

# Per-plane merged spec: lat = (c0, a, d): cells {c0+a*i+d*k, i<4, k<2},
# T-tile col = 2i+k.  usubs: U-op (q,e) iteration: out col, partner-read
# slot, tau(e=0) per q.  All offsets/steps in SLOT units (scaled by J).
PLANE_SPECS = {
    6: dict(lat=(0, 1, 4), usubs=[
        dict(u_off=4, u_dims=[[-4, 2], [1, 2]],
             xp_off=6, xp_dims=[[-2, 2], [-4, 2]], tau=(1, -1)),
        dict(u_off=2, u_dims=[[4, 2], [1, 2]],
             xp_off=5, xp_dims=[[2, 2], [-4, 2]], tau=(1, -1))]),
    9: dict(lat=(0, 4, 1), usubs=[
        dict(u_off=6, u_dims=[[-4, 2], [-1, 2]],
             xp_off=9, xp_dims=[[-8, 2], [3, 2]], tau=(1, -1)),
        dict(u_off=3, u_dims=[[4, 2], [-3, 2]],
             xp_off=0, xp_dims=[[8, 2], [5, 2]], tau=(1, -1))]),
    5: dict(lat=(4, 1, 6), usingle=dict(
        u_off=1, u_dims=[[2, 4], [-1, 2]],
        xp_off=4, xp_dims=[[1, 4], [6, 2]],
        ss_off=0, ss_dims=[[0, 4], [1, 2]]), usubs=[]),
    3: dict(lat=(0, 1, 10), usingle=dict(
        u_off=0, u_dims=[[2, 4], [1, 2]],
        xp_off=10, xp_dims=[[1, 4], [-10, 2]],
        ss_off=0, ss_dims=[[1, 4], [4, 2]]), usubs=[]),
}

# tables: per plane CC [J] + SS 4 rows [s,-s,-s,s]; stage order
_TBL_CC = {6: 0, 9: 5, 5: 10, 3: 15}
TBL_J = 24
# m3 SS has 8 rows: [t_q*s]_q + [-t_q*s]_q with t=(+,-,-,+)
_M3_TAU = (1.0, -1.0, -1.0, 1.0)

EARLY_OUT = (4, 10)                  # slots 4..9 final after stage 5


def _build_cayley(k=4):
    n = 1 << k
    C = np.zeros((n, n, n), dtype=np.float32)
    for a in range(n):
        for b in range(n):
            s, t = 0, a >> 1
            while t:
                s += bin(t & b).count("1")
                t >>= 1
            C[a, b, a ^ b] = -1.0 if (s & 1) else 1.0
    return C


def _verify_layout(cayley):
    """Re-derive every stage from SLOT/PLANE_SPECS and check against the
    runtime Cayley tensor via a tiny numeric simulation."""
    rng = np.random.default_rng(3)
    Jt = 8
    x = rng.standard_normal((MV, Jt))
    ang = rng.standard_normal((4, Jt))
    ref = x.copy()
    for si, m in enumerate(STAGE_ORDER):
        c2, s2 = np.cos(ang[si]), np.sin(ang[si])
        new = ref.copy()
        for a in range(MV):
            if bin(a & m).count("1") % 2 == 1:
                b = a ^ m
                new[a] = c2 * ref[a] + cayley[a, m, b] * s2 * ref[b]
        ref = new
    X = np.zeros((NSLOT, Jt))
    for c in COMPS:
        X[SLOT[c]] = x[c]
    for si, m in enumerate(STAGE_ORDER):
        c2, s2 = np.cos(ang[si]), np.sin(ang[si])
        sp = PLANE_SPECS[m]
        c0, a, d = sp["lat"]
        T = np.zeros((8, Jt))
        U = np.zeros((8, Jt))
        for i in range(4):
            for k in range(2):
                T[2 * i + k] = X[c0 + a * i + d * k] * c2
        if "usingle" in sp:
            us = sp["usingle"]
            taus = _M3_TAU if m == 3 else (1.0,) * 4
            for q in range(4):
                for e in range(2):
                    ucol = us["u_off"] + us["u_dims"][0][0] * q + \
                        us["u_dims"][1][0] * e
                    xs = us["xp_off"] + us["xp_dims"][0][0] * q + \
                        us["xp_dims"][1][0] * e
                    sgn = taus[q] * (1.0 if e == 0 else -1.0)
                    U[ucol] = X[xs] * sgn * s2
        for us in sp["usubs"]:
            for q in range(2):
                for e in range(2):
                    ucol = us["u_off"] + us["u_dims"][0][0] * q + \
                        us["u_dims"][1][0] * e
                    xs = us["xp_off"] + us["xp_dims"][0][0] * q + \
                        us["xp_dims"][1][0] * e
                    sgn = us["tau"][q] * (1.0 if e == 0 else -1.0)
                    U[ucol] = X[xs] * sgn * s2
        for i in range(4):
            for k in range(2):
                X[c0 + a * i + d * k] = T[2 * i + k] + U[2 * i + k]
    got = np.zeros((MV, Jt))
    got[0], got[15] = x[0], x[15]
    for c in COMPS:
        got[c] = X[SLOT[c]]
    assert np.abs(got - ref).max() < 1e-9, "layout/spec validation failed"


def _ap(base_ap, extra_off, dims):
    ap = [list(base_ap.ap[0])] + [list(d) for d in dims]
    return bass.AP(base_ap.tensor, base_ap.offset + extra_off, ap)


def _ss_dims(tau):
    """AP (offset_J, dims) into 4-row table [s,-s,-s,s] giving
    tau[q]*(-1)^e across (q,e)."""
    if tau == (1, 1):
        return 0, [[0, 2], [1, 2]]
    if tau == (-1, -1):
        return 1, [[0, 2], [-1, 2]]
    if tau == (1, -1):
        return 0, [[2, 2], [1, 2]]
    # (-1, 1): r = 1 - q + 2e
    return 1, [[-1, 2], [2, 2]]


def _build_program():
    nc = bacc.Bacc("TRN2", target_bir_lowering=False, debug=False,
                   enable_asserts=False, num_devices=NCORES)
    x_d = nc.dram_tensor("x", [P, NSLOT * J], F16, kind="ExternalInput")
    t_d = nc.dram_tensor("tbl", [P, TBL_J * J], F16, kind="ExternalInput")
    out_d = nc.dram_tensor("out", [P, NSLOT * J], F16, kind="ExternalOutput")

    with tile.TileContext(nc) as tc:
        with tc.tile_pool(name="data", bufs=1) as dpool, \
             tc.tile_pool(name="tu", bufs=1) as tupool:
            TBL = dpool.tile([P, TBL_J * J], F16)
            X = dpool.tile([P, NSLOT * J], F16)

            # stage-6 gate = its tables + x slots 0..7 only; the rest of x
            # and the later tables stream in under stage-6 compute
            nc.sync.dma_start(TBL[:, :5 * J], t_d[:, :5 * J])
            nc.sync.dma_start(X[:, :8 * J], x_d[:, :8 * J])
            nc.sync.dma_start(TBL[:, 5 * J:10 * J], t_d[:, 5 * J:10 * J])
            nc.sync.dma_start(X[:, 8 * J:], x_d[:, 8 * J:])
            nc.sync.dma_start(TBL[:, 10 * J:], t_d[:, 10 * J:])

            tu_dims = [[2 * J, 4], [J, 2], [1, J]]
            for m in STAGE_ORDER:
                sp = PLANE_SPECS[m]
                c0, a, d = sp["lat"]
                cc_j = _TBL_CC[m]
                grid = [[a * J, 4], [d * J, 2], [1, J]]
                T = tupool.tile([P, 8 * J], F16, tag="t")
                U = tupool.tile([P, 8 * J], F16, tag="u")
                nc.vector.tensor_mul(
                    _ap(T[:], 0, tu_dims),
                    _ap(X[:], c0 * J, grid),
                    _ap(TBL[:], cc_j * J, [[0, 4], [0, 2], [1, J]]))
                if "usingle" in sp:
                    us = sp["usingle"]
                    nc.vector.tensor_mul(
                        _ap(U[:], us["u_off"] * J,
                            [[us["u_dims"][0][0] * J, 4],
                             [us["u_dims"][1][0] * J, 2], [1, J]]),
                        _ap(X[:], us["xp_off"] * J,
                            [[us["xp_dims"][0][0] * J, 4],
                             [us["xp_dims"][1][0] * J, 2], [1, J]]),
                        _ap(TBL[:], (cc_j + 1 + us["ss_off"]) * J,
                            [[us["ss_dims"][0][0] * J, 4],
                             [us["ss_dims"][1][0] * J, 2], [1, J]]))
                for us in sp["usubs"]:
                    so, sd = _ss_dims(tuple(us["tau"]))
                    nc.vector.tensor_mul(
                        _ap(U[:], us["u_off"] * J,
                            [[us["u_dims"][0][0] * J, 2],
                             [us["u_dims"][1][0] * J, 2], [1, J]]),
                        _ap(X[:], us["xp_off"] * J,
                            [[us["xp_dims"][0][0] * J, 2],
                             [us["xp_dims"][1][0] * J, 2], [1, J]]),
                        _ap(TBL[:], (cc_j + 1 + so) * J,
                            [[sd[0][0] * J, 2], [sd[1][0] * J, 2], [1, J]]))
                if m == STAGE_ORDER[-1]:
                    # split add by k-halves; overlap output DMAs
                    for k in range(2):
                        nc.vector.tensor_add(
                            _ap(X[:], (c0 + d * k) * J,
                                [[a * J, 4], [1, J]]),
                            _ap(T[:], k * J, [[2 * J, 4], [1, J]]),
                            _ap(U[:], k * J, [[2 * J, 4], [1, J]]))
                        lo = c0 + d * k
                        nc.sync.dma_start(
                            out_d[:, lo * J:(lo + 4) * J],
                            X[:, lo * J:(lo + 4) * J])
                else:
                    nc.vector.tensor_add(
                        _ap(X[:], c0 * J, grid),
                        _ap(T[:], 0, tu_dims),
                        _ap(U[:], 0, tu_dims))
                if m == 5:
                    aa, bb = EARLY_OUT
                    nc.sync.dma_start(out_d[:, aa * J:bb * J],
                                      X[:, aa * J:bb * J])

    nc.compile()
    return nc


_PROGRAM_CACHE = {}


def _get_program():
    if "p" not in _PROGRAM_CACHE:
        _PROGRAM_CACHE["p"] = _build_program()
    return _PROGRAM_CACHE["p"]


def _build_in_maps(x, pos, coefs, theta0, cayley):
    """Host-side: slot-permuted fp16 x + per-core cos/sin tables."""
    _verify_layout(cayley)
    ang = theta0.astype(np.float64) * np.asarray(coefs, np.float64)[None, :]
    ctab = np.cos(ang).astype(np.float16)          # (MAX_LEN, 4)
    stab = np.sin(ang).astype(np.float16)
    plane_idx = {m: PLANE_BLADES.index(m) for m in STAGE_ORDER}

    pos_i = np.clip(pos, 0, MAX_LEN - 1).astype(np.int64)
    comp_order = [SLOT_TO_COMP[s] for s in range(NSLOT)]

    in_maps = []
    for g in range(NCORES):
        xr = np.ascontiguousarray(
            x[g * ROWS_PER_CORE:(g + 1) * ROWS_PER_CORE]
        ).reshape(P, J, MV)
        xs = xr.transpose(0, 2, 1)[:, comp_order, :]       # (P, 14, J)
        x16 = np.ascontiguousarray(xs).astype(np.float16).reshape(
            P, NSLOT * J)

        pg = pos_i[g * ROWS_PER_CORE:(g + 1) * ROWS_PER_CORE].reshape(P, J)
        tbl = np.empty((P, TBL_J, J), dtype=np.float16)
        for m in STAGE_ORDER:
            cc = _TBL_CC[m]
            i = plane_idx[m]
            c2 = ctab[pg, i]
            s2 = stab[pg, i]
            tbl[:, cc, :] = c2
            if m == 3:
                for q in range(4):
                    tbl[:, cc + 1 + q, :] = np.float16(_M3_TAU[q]) * s2
                    tbl[:, cc + 5 + q, :] = np.float16(-_M3_TAU[q]) * s2
            else:
                tbl[:, cc + 1, :] = s2
                tbl[:, cc + 2, :] = -s2
                tbl[:, cc + 3, :] = -s2
                tbl[:, cc + 4, :] = s2
        in_maps.append({"x": x16, "tbl": tbl.reshape(P, TBL_J * J)})
    return in_maps


def kernel(x, pos, bx, by, bz, bw, theta, cayley, biv_mask, scalar_mask):
    x = np.asarray(x, dtype=np.float32)
    pos = np.asarray(pos)
    theta = np.asarray(theta, dtype=np.float32)
    cayley = np.asarray(cayley, dtype=np.float32)
    assert x.shape == (B, L, MV) and pos.shape == (B, L)

    coefs = [float(np.asarray(c, dtype=np.float32).reshape(MV)[b])
             for c, b in zip((bx, by, bz, bw), PLANE_BLADES)]
    theta0 = theta.reshape(MAX_LEN, 4)

    nc = _get_program()
    in_maps = _build_in_maps(x, pos, coefs, theta0, cayley)
    res = run_bass_kernel_spmd(nc, in_maps, core_ids=list(range(NCORES)))

    out = np.empty((B, L, MV), dtype=np.float32)
    comp_order = [SLOT_TO_COMP[s] for s in range(NSLOT)]
    for g in range(NCORES):
        r = res.results[g]["out"].reshape(P, NSLOT, J).astype(np.float32)
        og = np.empty((P, MV, J), dtype=np.float32)
        og[:, comp_order, :] = r
        xr = np.ascontiguousarray(
            x[g * ROWS_PER_CORE:(g + 1) * ROWS_PER_CORE]).reshape(P, J, MV)
        og[:, 0, :] = xr[:, :, 0]
        og[:, 15, :] = xr[:, :, 15]
        out[g * ROWS_PER_CORE:(g + 1) * ROWS_PER_CORE] = \
            og.transpose(0, 2, 1).reshape(ROWS_PER_CORE, L, MV)
    return out


# revision 21
# speedup vs baseline: 1.0446x; 1.0110x over previous
"""Trainium2 Bass kernel for CARE position encoding (rotor sandwich).

out = R x R~ factorizes into 4 sequential Givens stages (blades 6,9,5,3).
Implementation highlights:
  - all cos/sin tables computed on the HOST and shipped fp16; the device
    does no transcendental work and never sees `pos`;
  - x stored per-core position-innermost: X[partition, slot*J + j]
    (J=256, 14 slots; multivector comps 0/15 are invariant -> host copy);
  - every stage = 4 fp16 DVE tensor_tensor ops (merged T multiply, two
    half U multiplies, merged add) whose innermost dims are 256-long
    unit-stride runs -> DVE 2x_1P packed mode;
  - slot permutation chosen so each plane's 8 rotated cells form a 2-dim
    slot lattice {c0+a*i+d*k} (one full-width T/A op) and stage 6's cells
    are exactly slots 0..7, so the x DMA splits into an early gating
    chunk and the rest overlaps stage-6 compute;
  - stage-3 (last) add is split by lattice k-halves so output DMAs start
    while the second half computes.
"""
import numpy as np

import concourse.bass as bass
import concourse.tile as tile
from concourse import bacc, mybir
from concourse.bass_utils import run_bass_kernel_spmd

F16 = mybir.dt.float16
F32 = mybir.dt.float32

P = 128
NCORES = 8
B, L, MV = 16, 16384, 16
MAX_LEN = 16384
ROWS_PER_CORE = B // NCORES          # 2
N = ROWS_PER_CORE * L                # 32768 positions per core
J = N // P                           # 256 positions per partition
NSLOT = 14

PLANE_BLADES = (3, 5, 9, 6)          # reference arg order
STAGE_ORDER = (6, 9, 5, 3)           # innermost rotor applied first

# slot[comp]; comps 0 and 15 bypass the device (host passthrough)
SLOT = {1: 12, 2: 2, 3: 4, 4: 6, 5: 0, 6: 10, 7: 8, 8: 9,
        9: 11, 10: 1, 11: 7, 12: 5, 13: 3, 14: 13}
COMPS = [c for c in range(MV) if c not in (0, 15)]
SLOT_TO_COMP = {s: c for c, s in SLOT.items()}

# Per-plane merged spec: lat = (c0, a, d): cells {c0+a*i+d*k, i<4, k<2},
# T-tile col = 2i+k.  usubs: U-op (q,e) iteration: out col, partner-read
# slot, tau(e=0) per q.  All offsets/steps in SLOT units (scaled by J).
PLANE_SPECS = {
    6: dict(lat=(0, 1, 4), usubs=[
        dict(u_off=4, u_dims=[[-4, 2], [1, 2]],
             xp_off=6, xp_dims=[[-2, 2], [-4, 2]], tau=(1, -1)),
        dict(u_off=2, u_dims=[[4, 2], [1, 2]],
             xp_off=5, xp_dims=[[2, 2], [-4, 2]], tau=(1, -1))]),
    9: dict(lat=(0, 4, 1), usubs=[
        dict(u_off=6, u_dims=[[-4, 2], [-1, 2]],
             xp_off=9, xp_dims=[[-8, 2], [3, 2]], tau=(1, -1)),
        dict(u_off=3, u_dims=[[4, 2], [-3, 2]],
             xp_off=0, xp_dims=[[8, 2], [5, 2]], tau=(1, -1))]),
    5: dict(lat=(4, 1, 6), usubs=[
        dict(u_off=5, u_dims=[[-4, 2], [-1, 2]],
             xp_off=6, xp_dims=[[-2, 2], [6, 2]], tau=(1, 1)),
        dict(u_off=3, u_dims=[[4, 2], [-1, 2]],
             xp_off=5, xp_dims=[[2, 2], [6, 2]], tau=(1, 1))]),
    3: dict(lat=(0, 1, 10), usubs=[
        dict(u_off=5, u_dims=[[-4, 2], [-1, 2]],
             xp_off=2, xp_dims=[[-2, 2], [10, 2]], tau=(1, -1)),
        dict(u_off=3, u_dims=[[4, 2], [-1, 2]],
             xp_off=1, xp_dims=[[2, 2], [10, 2]], tau=(1, -1))]),
}

# tables: per plane CC [J] + SS 4 rows [s,-s,-s,s]; stage order
_TBL_CC = {6: 0, 9: 5, 5: 10, 3: 15}
TBL_J = 20

EARLY_OUT = (4, 10)                  # slots 4..9 final after stage 5


def _build_cayley(k=4):
    n = 1 << k
    C = np.zeros((n, n, n), dtype=np.float32)
    for a in range(n):
        for b in range(n):
            s, t = 0, a >> 1
            while t:
                s += bin(t & b).count("1")
                t >>= 1
            C[a, b, a ^ b] = -1.0 if (s & 1) else 1.0
    return C


def _verify_layout(cayley):
    """Re-derive every stage from SLOT/PLANE_SPECS and check against the
    runtime Cayley tensor via a tiny numeric simulation."""
    rng = np.random.default_rng(3)
    Jt = 8
    x = rng.standard_normal((MV, Jt))
    ang = rng.standard_normal((4, Jt))
    ref = x.copy()
    for si, m in enumerate(STAGE_ORDER):
        c2, s2 = np.cos(ang[si]), np.sin(ang[si])
        new = ref.copy()
        for a in range(MV):
            if bin(a & m).count("1") % 2 == 1:
                b = a ^ m
                new[a] = c2 * ref[a] + cayley[a, m, b] * s2 * ref[b]
        ref = new
    X = np.zeros((NSLOT, Jt))
    for c in COMPS:
        X[SLOT[c]] = x[c]
    for si, m in enumerate(STAGE_ORDER):
        c2, s2 = np.cos(ang[si]), np.sin(ang[si])
        sp = PLANE_SPECS[m]
        c0, a, d = sp["lat"]
        T = np.zeros((8, Jt))
        U = np.zeros((8, Jt))
        for i in range(4):
            for k in range(2):
                T[2 * i + k] = X[c0 + a * i + d * k] * c2
        for us in sp["usubs"]:
            for q in range(2):
                for e in range(2):
                    ucol = us["u_off"] + us["u_dims"][0][0] * q + \
                        us["u_dims"][1][0] * e
                    xs = us["xp_off"] + us["xp_dims"][0][0] * q + \
                        us["xp_dims"][1][0] * e
                    sgn = us["tau"][q] * (1.0 if e == 0 else -1.0)
                    U[ucol] = X[xs] * sgn * s2
        for i in range(4):
            for k in range(2):
                X[c0 + a * i + d * k] = T[2 * i + k] + U[2 * i + k]
    got = np.zeros((MV, Jt))
    got[0], got[15] = x[0], x[15]
    for c in COMPS:
        got[c] = X[SLOT[c]]
    assert np.abs(got - ref).max() < 1e-9, "layout/spec validation failed"


def _ap(base_ap, extra_off, dims):
    ap = [list(base_ap.ap[0])] + [list(d) for d in dims]
    return bass.AP(base_ap.tensor, base_ap.offset + extra_off, ap)


def _ss_dims(tau):
    """AP (offset_J, dims) into 4-row table [s,-s,-s,s] giving
    tau[q]*(-1)^e across (q,e)."""
    if tau == (1, 1):
        return 0, [[0, 2], [1, 2]]
    if tau == (-1, -1):
        return 1, [[0, 2], [-1, 2]]
    if tau == (1, -1):
        return 0, [[2, 2], [1, 2]]
    # (-1, 1): r = 1 - q + 2e
    return 1, [[-1, 2], [2, 2]]


def _build_program():
    nc = bacc.Bacc("TRN2", target_bir_lowering=False, debug=False,
                   enable_asserts=False, num_devices=NCORES)
    x_d = nc.dram_tensor("x", [P, NSLOT * J], F16, kind="ExternalInput")
    t_d = nc.dram_tensor("tbl", [P, TBL_J * J], F16, kind="ExternalInput")
    out_d = nc.dram_tensor("out", [P, NSLOT * J], F16, kind="ExternalOutput")

    with tile.TileContext(nc) as tc:
        with tc.tile_pool(name="data", bufs=1) as dpool, \
             tc.tile_pool(name="tu", bufs=1) as tupool:
            TBL = dpool.tile([P, TBL_J * J], F16)
            X = dpool.tile([P, NSLOT * J], F16)

            # stage-6 gate = its tables + x slots 0..7 only; the rest of x
            # and the later tables stream in under stage-6 compute
            nc.sync.dma_start(TBL[:, :5 * J], t_d[:, :5 * J])
            nc.sync.dma_start(X[:, :8 * J], x_d[:, :8 * J])
            nc.sync.dma_start(X[:, 8 * J:], x_d[:, 8 * J:])
            nc.sync.dma_start(TBL[:, 5 * J:], t_d[:, 5 * J:])

            tu_dims = [[2 * J, 4], [J, 2], [1, J]]
            for m in STAGE_ORDER:
                sp = PLANE_SPECS[m]
                c0, a, d = sp["lat"]
                cc_j = _TBL_CC[m]
                grid = [[a * J, 4], [d * J, 2], [1, J]]
                T = tupool.tile([P, 8 * J], F16, tag="t")
                U = tupool.tile([P, 8 * J], F16, tag="u")
                nc.vector.tensor_mul(
                    _ap(T[:], 0, tu_dims),
                    _ap(X[:], c0 * J, grid),
                    _ap(TBL[:], cc_j * J, [[0, 4], [0, 2], [1, J]]))
                for us in sp["usubs"]:
                    so, sd = _ss_dims(tuple(us["tau"]))
                    nc.vector.tensor_mul(
                        _ap(U[:], us["u_off"] * J,
                            [[us["u_dims"][0][0] * J, 2],
                             [us["u_dims"][1][0] * J, 2], [1, J]]),
                        _ap(X[:], us["xp_off"] * J,
                            [[us["xp_dims"][0][0] * J, 2],
                             [us["xp_dims"][1][0] * J, 2], [1, J]]),
                        _ap(TBL[:], (cc_j + 1 + so) * J,
                            [[sd[0][0] * J, 2], [sd[1][0] * J, 2], [1, J]]))
                if m == STAGE_ORDER[-1]:
                    # split add by k-halves; overlap output DMAs
                    for k in range(2):
                        nc.vector.tensor_add(
                            _ap(X[:], (c0 + d * k) * J,
                                [[a * J, 4], [1, J]]),
                            _ap(T[:], k * J, [[2 * J, 4], [1, J]]),
                            _ap(U[:], k * J, [[2 * J, 4], [1, J]]))
                        lo = c0 + d * k
                        nc.sync.dma_start(
                            out_d[:, lo * J:(lo + 4) * J],
                            X[:, lo * J:(lo + 4) * J])
                else:
                    nc.vector.tensor_add(
                        _ap(X[:], c0 * J, grid),
                        _ap(T[:], 0, tu_dims),
                        _ap(U[:], 0, tu_dims))
                if m == 5:
                    aa, bb = EARLY_OUT
                    nc.sync.dma_start(out_d[:, aa * J:bb * J],
                                      X[:, aa * J:bb * J])

    nc.compile()
    return nc


_PROGRAM_CACHE = {}


def _get_program():
    if "p" not in _PROGRAM_CACHE:
        _PROGRAM_CACHE["p"] = _build_program()
    return _PROGRAM_CACHE["p"]


def _build_in_maps(x, pos, coefs, theta0, cayley):
    """Host-side: slot-permuted fp16 x + per-core cos/sin tables."""
    _verify_layout(cayley)
    ang = theta0.astype(np.float64) * np.asarray(coefs, np.float64)[None, :]
    ctab = np.cos(ang).astype(np.float16)          # (MAX_LEN, 4)
    stab = np.sin(ang).astype(np.float16)
    plane_idx = {m: PLANE_BLADES.index(m) for m in STAGE_ORDER}

    pos_i = np.clip(pos, 0, MAX_LEN - 1).astype(np.int64)
    comp_order = [SLOT_TO_COMP[s] for s in range(NSLOT)]

    in_maps = []
    for g in range(NCORES):
        xr = np.ascontiguousarray(
            x[g * ROWS_PER_CORE:(g + 1) * ROWS_PER_CORE]
        ).reshape(P, J, MV)
        xs = xr.transpose(0, 2, 1)[:, comp_order, :]       # (P, 14, J)
        x16 = np.ascontiguousarray(xs).astype(np.float16).reshape(
            P, NSLOT * J)

        pg = pos_i[g * ROWS_PER_CORE:(g + 1) * ROWS_PER_CORE].reshape(P, J)
        tbl = np.empty((P, TBL_J, J), dtype=np.float16)
        for m in STAGE_ORDER:
            cc = _TBL_CC[m]
            i = plane_idx[m]
            c2 = ctab[pg, i]
            s2 = stab[pg, i]
            tbl[:, cc, :] = c2
            tbl[:, cc + 1, :] = s2
            tbl[:, cc + 2, :] = -s2
            tbl[:, cc + 3, :] = -s2
            tbl[:, cc + 4, :] = s2
        in_maps.append({"x": x16, "tbl": tbl.reshape(P, TBL_J * J)})
    return in_maps


def kernel(x, pos, bx, by, bz, bw, theta, cayley, biv_mask, scalar_mask):
    x = np.asarray(x, dtype=np.float32)
    pos = np.asarray(pos)
    theta = np.asarray(theta, dtype=np.float32)
    cayley = np.asarray(cayley, dtype=np.float32)
    assert x.shape == (B, L, MV) and pos.shape == (B, L)

    coefs = [float(np.asarray(c, dtype=np.float32).reshape(MV)[b])
             for c, b in zip((bx, by, bz, bw), PLANE_BLADES)]
    theta0 = theta.reshape(MAX_LEN, 4)

    nc = _get_program()
    in_maps = _build_in_maps(x, pos, coefs, theta0, cayley)
    res = run_bass_kernel_spmd(nc, in_maps, core_ids=list(range(NCORES)))

    out = np.empty((B, L, MV), dtype=np.float32)
    comp_order = [SLOT_TO_COMP[s] for s in range(NSLOT)]
    for g in range(NCORES):
        r = res.results[g]["out"].reshape(P, NSLOT, J).astype(np.float32)
        og = np.empty((P, MV, J), dtype=np.float32)
        og[:, comp_order, :] = r
        xr = np.ascontiguousarray(
            x[g * ROWS_PER_CORE:(g + 1) * ROWS_PER_CORE]).reshape(P, J, MV)
        og[:, 0, :] = xr[:, :, 0]
        og[:, 15, :] = xr[:, :, 15]
        out[g * ROWS_PER_CORE:(g + 1) * ROWS_PER_CORE] = \
            og.transpose(0, 2, 1).reshape(ROWS_PER_CORE, L, MV)
    return out
